# revision 1
# baseline (speedup 1.0000x reference)
"""Trainium2 Bass kernel for 3-layer GraphSAGE (mean aggr) over 8 NeuronCores.

Strategy (hardcoded for N=50000, E=800000, F=128->256->256->10):
  - Nodes sharded across 8 cores: core c owns global nodes [c*6250,(c+1)*6250),
    padded locally to 6272 = 49 groups of 128.
  - Edges partitioned by destination owner; per core, edges are sorted by local
    dst and packed into chunks of 128 edges whose dst's lie within one 128-node
    group. Chunk counts per group are equalized across cores (pad edges) so a
    single SPMD program works for all cores.
  - Gather of source-node features: indirect DMA (row gather) from a replicated
    (layer 1) or all-gathered (layers 2/3) DRAM feature table.
  - Segment mean: per chunk, selection matrix S[e, col] = (dstcol[e]==col) *
    invdeg[e] built on DVE from a host iota and per-edge scalars; aggregation
    is matmul lhsT=G (edges x F), rhs=S -> PSUM [F, nodes] accumulated over the
    group's chunks (feature-major output feeds the dense matmuls directly).
  - Layer 3 pushes the Wl matmul *before* aggregation (linearity), so only a
    [N,16] table is gathered instead of [N,256].
  - Collectives: AllGather of h1 (row-major shard) and of p3 = h2 @ W3l.T.
"""

import os
import numpy as np

_P = 128
_N, _E, _FIN, _HID, _OUT, _OUTP = 50000, 800000, 128, 256, 10, 16
_C = 8
_NL = _N // _C            # 6250
_G = (_NL + _P - 1) // _P  # 49
_NLP = _G * _P            # 6272
_NGP = _C * _NLP          # 50176
_BG1, _BG2, _BG3 = 8, 4, 24   # gather sub-chunks per indirect DMA, per layer


def _prep(x, edge_index):
    """Host-side edge partitioning. Returns per-core arrays + chunk structure."""
    src = np.asarray(edge_index[0], dtype=np.int64)
    dst = np.asarray(edge_index[1], dtype=np.int64)
    owner = dst // _NL
    dl = (dst - owner * _NL).astype(np.int64)
    # source index in the padded global layout used by h1_full / p3_full / x_pad
    srcp = ((src // _NL) * _NLP + (src % _NL)).astype(np.int64)

    per_core = []
    gdeg = np.zeros((_C, _G), dtype=np.int64)
    for c in range(_C):
        m = owner == c
        s_c, d_c = srcp[m], dl[m]
        order = np.argsort(d_c, kind="stable")
        s_c, d_c = s_c[order], d_c[order]
        deg = np.bincount(d_c, minlength=_NLP)
        gdeg[c] = deg.reshape(_G, _P).sum(1)
        per_core.append((s_c, d_c, deg))

    chunks_g = np.maximum(1, np.ceil(gdeg.max(0) / _P).astype(np.int64))  # [G]
    T = int(chunks_g.sum())
    cstart = np.concatenate([[0], np.cumsum(chunks_g)]).astype(np.int64)

    maps = []
    for c in range(_C):
        s_c, d_c, deg = per_core[c]
        invdeg = (1.0 / np.maximum(deg, 1)).astype(np.float32)
        offs = np.zeros((T, _P), np.int32)
        dcol = np.full((T, _P), -1.0, np.float32)
        ivd = np.zeros((T, _P), np.float32)
        bounds = np.searchsorted(d_c, np.arange(_G + 1) * _P, "left")
        for g in range(_G):
            lo, hi = bounds[g], bounds[g + 1]
            cnt = hi - lo
            tg = int(chunks_g[g])
            fo = np.zeros(tg * _P, np.int32)
            fo[:cnt] = s_c[lo:hi]
            fd = np.full(tg * _P, -1.0, np.float32)
            fd[:cnt] = (d_c[lo:hi] - g * _P).astype(np.float32)
            fv = np.zeros(tg * _P, np.float32)
            fv[:cnt] = invdeg[d_c[lo:hi]]
            t0 = int(cstart[g])
            offs[t0 : t0 + tg] = fo.reshape(tg, _P)
            dcol[t0 : t0 + tg] = fd.reshape(tg, _P)
            ivd[t0 : t0 + tg] = fv.reshape(tg, _P)
        xT = np.zeros((_FIN, _NLP), np.float32)
        xT[:, :_NL] = np.asarray(x[c * _NL : (c + 1) * _NL], np.float32).T
        maps.append(
            dict(
                offs=np.ascontiguousarray(offs.T),     # [128, T] int32
                dcol=np.ascontiguousarray(dcol.T),     # [128, T] f32
                ivd=np.ascontiguousarray(ivd.T),       # [128, T] f32
                xT=xT,
            )
        )
    return maps, T, chunks_g, cstart


def _build(T, chunks_g, cstart):
    """Build the SPMD Bass program. Returns (nc, input_names)."""
    import sys

    if "/opt/trn_rl_repo" not in sys.path:
        sys.path.insert(0, "/opt/trn_rl_repo")
    from concourse import bass, mybir, bacc
    import concourse.tile as tile

    f32 = mybir.dt.float32
    i32 = mybir.dt.int32
    Alu = mybir.AluOpType
    Act = mybir.ActivationFunctionType
    AxX = mybir.AxisListType.X

    nc = bacc.Bacc(
        "TRN2",
        target_bir_lowering=False,
        debug=False,
        enable_asserts=False,
        num_devices=_C,
    )

    # kernel I/O
    x_d = nc.dram_tensor("xpad", [_NGP, _FIN], f32, kind="ExternalInput")
    xT_d = nc.dram_tensor("xT", [_P, _NLP], f32, kind="ExternalInput")
    offs_d = nc.dram_tensor("offs", [_P, T], i32, kind="ExternalInput")
    dcol_d = nc.dram_tensor("dcol", [_P, T], f32, kind="ExternalInput")
    ivd_d = nc.dram_tensor("ivd", [_P, T], f32, kind="ExternalInput")
    iota_d = nc.dram_tensor("iota", [_P, _P], f32, kind="ExternalInput")
    ident_d = nc.dram_tensor("ident", [_P, _P], f32, kind="ExternalInput")
    w1l_d = nc.dram_tensor("w1lT", [_FIN, _HID], f32, kind="ExternalInput")
    w1r_d = nc.dram_tensor("w1rT", [_FIN, _HID], f32, kind="ExternalInput")
    w2l_d = nc.dram_tensor("w2lT", [_HID, _HID], f32, kind="ExternalInput")
    w2r_d = nc.dram_tensor("w2rT", [_HID, _HID], f32, kind="ExternalInput")
    w3l_d = nc.dram_tensor("w3lT", [_HID, _OUTP], f32, kind="ExternalInput")
    w3r_d = nc.dram_tensor("w3rT", [_HID, _OUTP], f32, kind="ExternalInput")
    b1_d = nc.dram_tensor("b1", [_P, 2], f32, kind="ExternalInput")
    b2_d = nc.dram_tensor("b2", [_P, 2], f32, kind="ExternalInput")
    b3_d = nc.dram_tensor("b3", [_P, _OUTP], f32, kind="ExternalInput")
    out_d = nc.dram_tensor("out", [_NLP, _OUTP], f32, kind="ExternalOutput")
    debug = os.environ.get("KDBG", "0") == "1"
    if debug:
        h1dbg_d = nc.dram_tensor("h1dbg", [_NLP, _HID], f32, kind="ExternalOutput")
        p3dbg_d = nc.dram_tensor("p3dbg", [_NLP, _OUTP], f32, kind="ExternalOutput")

    input_names = [
        "xpad", "xT", "offs", "dcol", "ivd", "iota", "ident",
        "w1lT", "w1rT", "w2lT", "w2rT", "w3lT", "w3rT", "b1", "b2", "b3",
    ]

    rg = [list(range(_C))]

    with tile.TileContext(nc) as tc:
        with (
            tc.tile_pool(name="dram", bufs=1, space="DRAM") as dp,
            tc.tile_pool(name="const", bufs=1) as cp,
            tc.tile_pool(name="gat", bufs=3) as gp,
            tc.tile_pool(name="sel", bufs=6) as sp,
            tc.tile_pool(name="work", bufs=4) as wp,
            tc.tile_pool(name="psA", bufs=2, space="PSUM") as psA,
            tc.tile_pool(name="psB", bufs=2, space="PSUM") as psB,
            tc.tile_pool(name="psT", bufs=2, space="PSUM") as psT,
        ):
            # DRAM scratch
            h1_shard = dp.tile([_NLP, _HID], f32, name="h1_shard")
            h1_full = dp.tile([_NGP, _HID], f32, name="h1_full",
                              addr_space="Shared")
            p3_shard = dp.tile([_NLP, _OUTP], f32, name="p3_shard")
            p3_full = dp.tile([_NGP, _OUTP], f32, name="p3_full",
                              addr_space="Shared")

            # ---- resident constants ----
            def load(dram, shape, dtype=f32, name=None):
                t = cp.tile(shape, dtype, name=name)
                nc.sync.dma_start(out=t[:], in_=dram.ap())
                return t

            offsT = load(offs_d, [_P, T], i32, "offsT")
            dcolT = load(dcol_d, [_P, T], f32, "dcolT")
            ivdT = load(ivd_d, [_P, T], f32, "ivdT")
            iota = load(iota_d, [_P, _P], f32, "iotaT")
            ident = load(ident_d, [_P, _P], f32, "identT")
            xT = load(xT_d, [_P, _NLP], f32, "xTt")
            w1l = load(w1l_d, [_FIN, _HID], f32, "w1lTt")
            w1r = load(w1r_d, [_FIN, _HID], f32, "w1rTt")
            w2lt, w2rt, w3lt, w3rt = [], [], [], []
            for f in range(2):
                t = cp.tile([_P, _HID], f32, name=f"w2l{f}")
                nc.sync.dma_start(out=t[:], in_=w2l_d.ap()[f * _P : (f + 1) * _P, :])
                w2lt.append(t)
                t = cp.tile([_P, _HID], f32, name=f"w2r{f}")
                nc.sync.dma_start(out=t[:], in_=w2r_d.ap()[f * _P : (f + 1) * _P, :])
                w2rt.append(t)
                t = cp.tile([_P, _OUTP], f32, name=f"w3l{f}")
                nc.sync.dma_start(out=t[:], in_=w3l_d.ap()[f * _P : (f + 1) * _P, :])
                w3lt.append(t)
                t = cp.tile([_P, _OUTP], f32, name=f"w3r{f}")
                nc.sync.dma_start(out=t[:], in_=w3r_d.ap()[f * _P : (f + 1) * _P, :])
                w3rt.append(t)
            b1 = load(b1_d, [_P, 2], f32, "b1t")
            b2 = load(b2_d, [_P, 2], f32, "b2t")
            b3 = load(b3_d, [_P, _OUTP], f32, "b3t")
            h1T = [cp.tile([_P, _NLP], f32, name=f"h1T{f}") for f in range(2)]
            h2T = [cp.tile([_P, _NLP], f32, name=f"h2T{f}") for f in range(2)]

            def make_S(t):
                S = sp.tile([_P, _P], f32, tag="S")
                nc.vector.tensor_scalar(
                    out=S[:], in0=iota[:],
                    scalar1=dcolT[:, t : t + 1], scalar2=ivdT[:, t : t + 1],
                    op0=Alu.is_equal, op1=Alu.mult,
                )
                return S

            def gather_group(g, src_ap, F, BG, tag):
                """Indirect-gather all chunks of group g; returns list of
                (chunk_index_global, sbuf_tile, free_offset)."""
                t0, tg = int(cstart[g]), int(chunks_g[g])
                out = []
                for b0 in range(0, tg, BG):
                    nb = min(BG, tg - b0)
                    gt = gp.tile([_P, nb * F], f32, tag=tag)
                    nc.gpsimd.indirect_dma_start(
                        out=gt[:],
                        out_offset=None,
                        in_=src_ap,
                        in_offset=bass.IndirectOffsetOnAxis(
                            ap=offsT[:, t0 + b0 : t0 + b0 + nb], axis=0
                        ),
                    )
                    for j in range(nb):
                        out.append((t0 + b0 + j, gt, j * F))
                return out

            # ================= Layer 1 =================
            for g in range(_G):
                t0, tg = int(cstart[g]), int(chunks_g[g])
                gl = gather_group(g, x_d.ap(), _FIN, _BG1, "g1")
                pa = psA.tile([_P, _P], f32, tag="agg")
                for k, (t, gt, fo) in enumerate(gl):
                    S = make_S(t)
                    nc.tensor.matmul(
                        out=pa[:], lhsT=gt[:, fo : fo + _FIN], rhs=S[:],
                        start=(k == 0), stop=(k == tg - 1),
                    )
                mean = wp.tile([_P, _P], f32, tag="mean")
                nc.vector.tensor_copy(out=mean[:], in_=pa[:])
                ns = slice(g * _P, (g + 1) * _P)
                for h in range(2):
                    hs = slice(h * _P, (h + 1) * _P)
                    ph = psB.tile([_P, _P], f32, tag="dense")
                    nc.tensor.matmul(out=ph[:], lhsT=w1l[:, hs], rhs=mean[:],
                                     start=True, stop=False)
                    nc.tensor.matmul(out=ph[:], lhsT=w1r[:, hs], rhs=xT[:, ns],
                                     start=False, stop=True)
                    nc.scalar.activation(out=h1T[h][:, ns], in_=ph[:],
                                         func=Act.Relu, bias=b1[:, h : h + 1])
                row = wp.tile([_P, _HID], f32, tag="row")
                for h in range(2):
                    pt = psT.tile([_P, _P], f32, tag="tr")
                    nc.tensor.transpose(out=pt[:], in_=h1T[h][:, ns],
                                        identity=ident[:])
                    nc.vector.tensor_copy(out=row[:, h * _P : (h + 1) * _P],
                                          in_=pt[:])
                nc.sync.dma_start(out=h1_shard[ns, :], in_=row[:])
                if debug:
                    nc.sync.dma_start(out=h1dbg_d.ap()[ns, :], in_=row[:])

            nc.gpsimd.collective_compute(
                "AllGather", Alu.bypass, replica_groups=rg,
                ins=[h1_shard.opt()], outs=[h1_full.opt()],
            )

            # ================= Layer 2 =================
            for g in range(_G):
                t0, tg = int(cstart[g]), int(chunks_g[g])
                gl = gather_group(g, h1_full[:], _HID, _BG2, "g2")
                pa = [psA.tile([_P, _P], f32, tag="agg", name="pa0"),
                      psT.tile([_P, _P], f32, tag="tr", name="pa1")]
                for k, (t, gt, fo) in enumerate(gl):
                    S = make_S(t)
                    for f in range(2):
                        nc.tensor.matmul(
                            out=pa[f][:],
                            lhsT=gt[:, fo + f * _P : fo + (f + 1) * _P],
                            rhs=S[:], start=(k == 0), stop=(k == tg - 1),
                        )
                mean = [wp.tile([_P, _P], f32, tag="mean", name="mean0"),
                        wp.tile([_P, _P], f32, tag="mean2", name="mean1")]
                for f in range(2):
                    nc.vector.tensor_copy(out=mean[f][:], in_=pa[f][:])
                ns = slice(g * _P, (g + 1) * _P)
                for h in range(2):
                    hs = slice(h * _P, (h + 1) * _P)
                    ph = psB.tile([_P, _P], f32, tag="dense")
                    for f in range(2):
                        nc.tensor.matmul(out=ph[:], lhsT=w2lt[f][:, hs],
                                         rhs=mean[f][:], start=(f == 0),
                                         stop=False)
                    for f in range(2):
                        nc.tensor.matmul(out=ph[:], lhsT=w2rt[f][:, hs],
                                         rhs=h1T[f][:, ns], start=False,
                                         stop=(f == 1))
                    nc.scalar.activation(out=h2T[h][:, ns], in_=ph[:],
                                         func=Act.Relu, bias=b2[:, h : h + 1])
                # p3 = h2 @ W3l.T  (row-major directly)
                pp = psA.tile([_P, _OUTP], f32, tag="p3")
                for f in range(2):
                    nc.tensor.matmul(out=pp[:], lhsT=h2T[f][:, ns],
                                     rhs=w3lt[f][:], start=(f == 0),
                                     stop=(f == 1))
                p3row = wp.tile([_P, _OUTP], f32, tag="p3row")
                nc.vector.tensor_copy(out=p3row[:], in_=pp[:])
                nc.sync.dma_start(out=p3_shard[ns, :], in_=p3row[:])
                if debug:
                    nc.sync.dma_start(out=p3dbg_d.ap()[ns, :], in_=p3row[:])

            nc.gpsimd.collective_compute(
                "AllGather", Alu.bypass, replica_groups=rg,
                ins=[p3_shard.opt()], outs=[p3_full.opt()],
            )

            # ================= Layer 3 + log_softmax =================
            for g in range(_G):
                t0, tg = int(cstart[g]), int(chunks_g[g])
                gl = gather_group(g, p3_full[:], _OUTP, _BG3, "g3")
                po = psB.tile([_P, _OUTP], f32, tag="dense")
                for k, (t, gt, fo) in enumerate(gl):
                    S = make_S(t)
                    nc.tensor.matmul(out=po[:], lhsT=S[:],
                                     rhs=gt[:, fo : fo + _OUTP],
                                     start=(k == 0), stop=False)
                ns = slice(g * _P, (g + 1) * _P)
                for f in range(2):
                    nc.tensor.matmul(out=po[:], lhsT=h2T[f][:, ns],
                                     rhs=w3rt[f][:], start=False, stop=(f == 1))
                z = wp.tile([_P, _OUTP], f32, tag="z")
                nc.vector.tensor_tensor(out=z[:], in0=po[:], in1=b3[:],
                                        op=Alu.add)
                mx = sp.tile([_P, 1], f32, tag="mx")
                nc.vector.reduce_max(mx[:], z[:], axis=AxX)
                zc = wp.tile([_P, _OUTP], f32, tag="zc")
                nc.vector.tensor_scalar(out=zc[:], in0=z[:], scalar1=mx[:],
                                        scalar2=None, op0=Alu.subtract)
                ez = wp.tile([_P, _OUTP], f32, tag="ez")
                nc.scalar.activation(out=ez[:], in_=zc[:], func=Act.Exp)
                sm = sp.tile([_P, 1], f32, tag="sm")
                nc.vector.reduce_sum(sm[:], ez[:], axis=AxX)
                lg = sp.tile([_P, 1], f32, tag="lg")
                nc.scalar.activation(out=lg[:], in_=sm[:], func=Act.Ln)
                res = wp.tile([_P, _OUTP], f32, tag="res")
                nc.vector.tensor_scalar(out=res[:], in0=zc[:], scalar1=lg[:],
                                        scalar2=None, op0=Alu.subtract)
                nc.sync.dma_start(out=out_d.ap()[ns, :], in_=res[:])

    nc.compile()
    return nc, input_names


def _run(inputs, trace=False, tmpdir=None):
    import sys

    if "/opt/trn_rl_repo" not in sys.path:
        sys.path.insert(0, "/opt/trn_rl_repo")
    from concourse import bass_utils

    x = np.asarray(inputs["x"], np.float32)
    maps, T, chunks_g, cstart = _prep(x, inputs["edge_index"])

    x_pad = np.zeros((_NGP, _FIN), np.float32)
    for c in range(_C):
        x_pad[c * _NLP : c * _NLP + _NL] = x[c * _NL : (c + 1) * _NL]

    iota = np.tile(np.arange(_P, dtype=np.float32), (_P, 1))
    ident = np.eye(_P, dtype=np.float32)
    w1lT = np.ascontiguousarray(np.asarray(inputs["W1l"], np.float32).T)
    w1rT = np.ascontiguousarray(np.asarray(inputs["W1r"], np.float32).T)
    w2lT = np.ascontiguousarray(np.asarray(inputs["W2l"], np.float32).T)
    w2rT = np.ascontiguousarray(np.asarray(inputs["W2r"], np.float32).T)
    w3lT = np.zeros((_HID, _OUTP), np.float32)
    w3lT[:, :_OUT] = np.asarray(inputs["W3l"], np.float32).T
    w3rT = np.zeros((_HID, _OUTP), np.float32)
    w3rT[:, :_OUT] = np.asarray(inputs["W3r"], np.float32).T
    b1 = np.ascontiguousarray(
        np.asarray(inputs["b1l"], np.float32).reshape(2, _P).T)
    b2 = np.ascontiguousarray(
        np.asarray(inputs["b2l"], np.float32).reshape(2, _P).T)
    b3 = np.full((_P, _OUTP), -1e9, np.float32)
    b3[:, :_OUT] = np.asarray(inputs["b3l"], np.float32)[None, :]

    shared = dict(
        xpad=x_pad, iota=iota, ident=ident,
        w1lT=w1lT, w1rT=w1rT, w2lT=w2lT, w2rT=w2rT, w3lT=w3lT, w3rT=w3rT,
        b1=b1, b2=b2, b3=b3,
    )
    in_maps = []
    for c in range(_C):
        m = dict(shared)
        m["xT"] = maps[c]["xT"]
        m["offs"] = maps[c]["offs"]
        m["dcol"] = maps[c]["dcol"]
        m["ivd"] = maps[c]["ivd"]
        in_maps.append(m)

    nc, input_names = _build(T, chunks_g, cstart)

    res = bass_utils.run_bass_kernel_spmd(
        nc, in_maps, core_ids=list(range(_C)), trace=trace, tmpdir=tmpdir,
    )
    outs = res.results
    y = np.concatenate(
        [np.asarray(outs[c]["out"])[:_NL, :_OUT] for c in range(_C)], axis=0
    ).astype(np.float32)
    return y, res


def kernel(**inputs):
    y, _ = _run(inputs, trace=False)
    return y



# revision 2
# speedup vs baseline: 2.8600x; 2.8600x over previous
"""Trainium2 Bass kernel for 3-layer GraphSAGE (mean aggr) over 8 NeuronCores.

Strategy (hardcoded for N=50000, E=800000, F=128->256->256->10):
  - Nodes sharded across 8 cores: core c owns global nodes [c*6250,(c+1)*6250),
    padded locally to 6272 = 49 groups of 128.
  - Edges partitioned by destination owner; per core, edges are sorted by local
    dst and packed into chunks of 128 edges whose dst's lie within one 128-node
    group. Chunk counts per group are equalized across cores (pad edges) so a
    single SPMD program works for all cores.
  - All matmul operands are fp16 (PSUM accumulation fp32).
  - Gather of source-node features: one indirect DMA (row gather) per group
    from a replicated (layer 1) or all-gathered (layers 2/3) fp16 DRAM table.
  - Segment mean: per group, a single batched DVE is_equal builds the binary
    selection matrix S[e, tg*128] from a host iota and per-edge dst columns;
    aggregation is matmul lhsT=G (edges x F), rhs=S chunk -> PSUM [F, nodes]
    accumulated over the group's chunks. 1/deg is applied by the PSUM->SBUF
    copy (tensor_tensor mult with a host-replicated invdeg tile), keeping S
    binary and cheap.
  - Layer 3 pushes the Wl matmul *before* aggregation (linearity), so only a
    [N,16] fp16 table is gathered instead of [N,256]. Bias b3 enters via a
    rank-1 matmul; log_softmax runs batched over all groups (single Exp/Ln
    table load).
  - Collectives: AllGather of h1 (fp16) and of p3 = h2 @ W3l.T (fp16).
"""

import os
import numpy as np

_P = 128
_N, _E, _FIN, _HID, _OUT, _OUTP = 50000, 800000, 128, 256, 10, 16
_C = 8
_NL = _N // _C            # 6250
_G = (_NL + _P - 1) // _P  # 49
_NLP = _G * _P            # 6272
_NGP = _C * _NLP          # 50176


def _prep(x, edge_index):
    """Host-side edge partitioning. Returns per-core arrays + chunk structure."""
    src = np.asarray(edge_index[0], dtype=np.int64)
    dst = np.asarray(edge_index[1], dtype=np.int64)
    owner = dst // _NL
    dl = (dst - owner * _NL).astype(np.int64)
    # source index in the padded global layout used by h1_full / p3_full / x_pad
    srcp = ((src // _NL) * _NLP + (src % _NL)).astype(np.int64)

    per_core = []
    gdeg = np.zeros((_C, _G), dtype=np.int64)
    for c in range(_C):
        m = owner == c
        s_c, d_c = srcp[m], dl[m]
        order = np.argsort(d_c, kind="stable")
        s_c, d_c = s_c[order], d_c[order]
        deg = np.bincount(d_c, minlength=_NLP)
        gdeg[c] = deg.reshape(_G, _P).sum(1)
        per_core.append((s_c, d_c, deg))

    chunks_g = np.maximum(1, np.ceil(gdeg.max(0) / _P).astype(np.int64))  # [G]
    T = int(chunks_g.sum())
    cstart = np.concatenate([[0], np.cumsum(chunks_g)]).astype(np.int64)

    maps = []
    for c in range(_C):
        s_c, d_c, deg = per_core[c]
        invdeg = (1.0 / np.maximum(deg, 1)).astype(np.float32)
        offs = np.zeros((T, _P), np.int32)
        dcol = np.full((T, _P), -1.0, np.float16)
        bounds = np.searchsorted(d_c, np.arange(_G + 1) * _P, "left")
        for g in range(_G):
            lo, hi = bounds[g], bounds[g + 1]
            cnt = hi - lo
            tg = int(chunks_g[g])
            fo = np.zeros(tg * _P, np.int32)
            fo[:cnt] = s_c[lo:hi]
            fd = np.full(tg * _P, -1.0, np.float16)
            fd[:cnt] = (d_c[lo:hi] - g * _P).astype(np.float16)
            t0 = int(cstart[g])
            offs[t0 : t0 + tg] = fo.reshape(tg, _P)
            dcol[t0 : t0 + tg] = fd.reshape(tg, _P)
        xT = np.zeros((_P, _NLP), np.float16)
        xT[:, :_NL] = np.asarray(x[c * _NL : (c + 1) * _NL], np.float32).T
        # invdeg replicated across 128 partitions (for the mean multiply)
        ivdbc = np.tile(invdeg[None, :], (_P, 1)).astype(np.float16)
        # invdeg in [node-within-group (partition), group] layout for layer 3
        ivdg = np.ascontiguousarray(
            invdeg.reshape(_G, _P).T).astype(np.float32)
        maps.append(
            dict(
                offs=np.ascontiguousarray(offs.T),     # [128, T] int32
                dcol=np.ascontiguousarray(dcol.T),     # [128, T] f16
                ivdbc=ivdbc,                           # [128, NLP] f16
                ivdg=ivdg,                             # [128, G] f32
                xT=xT,                                 # [128, NLP] f16
            )
        )
    return maps, T, chunks_g, cstart


def _build(T, chunks_g, cstart):
    """Build the SPMD Bass program. Returns (nc, input_names)."""
    import sys

    if "/opt/trn_rl_repo" not in sys.path:
        sys.path.insert(0, "/opt/trn_rl_repo")
    from concourse import bass, mybir, bacc
    import concourse.tile as tile

    f32 = mybir.dt.float32
    f16 = mybir.dt.float16
    i32 = mybir.dt.int32
    Alu = mybir.AluOpType
    Act = mybir.ActivationFunctionType
    AxX = mybir.AxisListType.X

    TGMAX = int(chunks_g.max())

    nc = bacc.Bacc(
        "TRN2",
        target_bir_lowering=False,
        debug=False,
        enable_asserts=False,
        num_devices=_C,
    )

    # kernel I/O
    x_d = nc.dram_tensor("xpad", [_NGP, _FIN], f16, kind="ExternalInput")
    xT_d = nc.dram_tensor("xT", [_P, _NLP], f16, kind="ExternalInput")
    offs_d = nc.dram_tensor("offs", [_P, T], i32, kind="ExternalInput")
    dcol_d = nc.dram_tensor("dcol", [_P, T], f16, kind="ExternalInput")
    ivdbc_d = nc.dram_tensor("ivdbc", [_P, _NLP], f16, kind="ExternalInput")
    ivdg_d = nc.dram_tensor("ivdg", [_P, _G], f32, kind="ExternalInput")
    iota_d = nc.dram_tensor("iota", [_P, _P], f16, kind="ExternalInput")
    ident_d = nc.dram_tensor("ident", [_P, _P], f16, kind="ExternalInput")
    ones_d = nc.dram_tensor("ones1", [1, _P], f16, kind="ExternalInput")
    w1l_d = nc.dram_tensor("w1lT", [_FIN, _HID], f16, kind="ExternalInput")
    w1r_d = nc.dram_tensor("w1rT", [_FIN, _HID], f16, kind="ExternalInput")
    w2l_d = nc.dram_tensor("w2lT", [_HID, _HID], f16, kind="ExternalInput")
    w2r_d = nc.dram_tensor("w2rT", [_HID, _HID], f16, kind="ExternalInput")
    w3l_d = nc.dram_tensor("w3lT", [_HID, _OUTP], f16, kind="ExternalInput")
    w3r_d = nc.dram_tensor("w3rT", [_HID, _OUTP], f16, kind="ExternalInput")
    b1_d = nc.dram_tensor("b1", [_P, 2], f32, kind="ExternalInput")
    b2_d = nc.dram_tensor("b2", [_P, 2], f32, kind="ExternalInput")
    b3_d = nc.dram_tensor("b3row", [1, _OUTP], f16, kind="ExternalInput")
    out_d = nc.dram_tensor("out", [_NLP, _OUTP], f32, kind="ExternalOutput")

    input_names = [
        "xpad", "xT", "offs", "dcol", "ivdbc", "ivdg", "iota", "ident",
        "ones1", "w1lT", "w1rT", "w2lT", "w2rT", "w3lT", "w3rT",
        "b1", "b2", "b3row",
    ]

    rg = [list(range(_C))]

    with tile.TileContext(nc) as tc:
        with (
            tc.tile_pool(name="dram", bufs=1, space="DRAM") as dp,
            tc.tile_pool(name="const", bufs=1) as cp,
            tc.tile_pool(name="gat", bufs=3) as gp,
            tc.tile_pool(name="sel", bufs=3) as sp,
            tc.tile_pool(name="work", bufs=4) as wp,
            tc.tile_pool(name="psA", bufs=2, space="PSUM") as psA,
            tc.tile_pool(name="psB", bufs=2, space="PSUM") as psB,
            tc.tile_pool(name="psT", bufs=2, space="PSUM") as psT,
            tc.tile_pool(name="psS", bufs=2, space="PSUM") as psS,
        ):
            # DRAM scratch
            h1_shard = dp.tile([_NLP, _HID], f16, name="h1_shard")
            h1_full = dp.tile([_NGP, _HID], f16, name="h1_full",
                              addr_space="Shared")
            p3_shard = dp.tile([_NLP, _OUTP], f16, name="p3_shard")
            p3_full = dp.tile([_NGP, _OUTP], f16, name="p3_full",
                              addr_space="Shared")

            # ---- resident constants ----
            def load(dram, shape, dtype, name):
                t = cp.tile(shape, dtype, name=name)
                nc.sync.dma_start(out=t[:], in_=dram.ap())
                return t

            offsT = load(offs_d, [_P, T], i32, "offsT")
            dcolT = load(dcol_d, [_P, T], f16, "dcolT")
            ivdbc = load(ivdbc_d, [_P, _NLP], f16, "ivdbcT")
            ivdg = load(ivdg_d, [_P, _G], f32, "ivdgT")
            iota = load(iota_d, [_P, _P], f16, "iotaT")
            ident = load(ident_d, [_P, _P], f16, "identT")
            ones1 = load(ones_d, [1, _P], f16, "ones1T")
            xT = load(xT_d, [_P, _NLP], f16, "xTt")
            w1l = load(w1l_d, [_FIN, _HID], f16, "w1lTt")
            w1r = load(w1r_d, [_FIN, _HID], f16, "w1rTt")
            w2lt, w2rt, w3lt, w3rt = [], [], [], []
            for f in range(2):
                t = cp.tile([_P, _HID], f16, name=f"w2l{f}")
                nc.sync.dma_start(out=t[:], in_=w2l_d.ap()[f * _P : (f + 1) * _P, :])
                w2lt.append(t)
                t = cp.tile([_P, _HID], f16, name=f"w2r{f}")
                nc.sync.dma_start(out=t[:], in_=w2r_d.ap()[f * _P : (f + 1) * _P, :])
                w2rt.append(t)
                t = cp.tile([_P, _OUTP], f16, name=f"w3l{f}")
                nc.sync.dma_start(out=t[:], in_=w3l_d.ap()[f * _P : (f + 1) * _P, :])
                w3lt.append(t)
                t = cp.tile([_P, _OUTP], f16, name=f"w3r{f}")
                nc.sync.dma_start(out=t[:], in_=w3r_d.ap()[f * _P : (f + 1) * _P, :])
                w3rt.append(t)
            b1 = load(b1_d, [_P, 2], f32, "b1t")
            b2 = load(b2_d, [_P, 2], f32, "b2t")
            b3row = load(b3_d, [1, _OUTP], f16, "b3t")
            h1T = [cp.tile([_P, _NLP], f16, name=f"h1T{f}") for f in range(2)]
            h2T = [cp.tile([_P, _NLP], f16, name=f"h2T{f}") for f in range(2)]
            zbuf = cp.tile([_P, _G * _OUTP], f32, name="zbuf")

            def make_S(g):
                """Batched binary selection matrix for all chunks of group g:
                S[e, k*128+c] = (dcol[e, t0+k] == c), one DVE op."""
                t0, tg = int(cstart[g]), int(chunks_g[g])
                S = sp.tile([_P, TGMAX * _P], f16, tag="S")
                iota_b = iota[:].unsqueeze(1).broadcast_to([_P, tg, _P])
                dcol_b = (
                    dcolT[:, t0 : t0 + tg].unsqueeze(2)
                    .broadcast_to([_P, tg, _P])
                )
                Sv = S[:, : tg * _P].rearrange("p (k c) -> p k c", k=tg)
                nc.vector.tensor_tensor(
                    out=Sv, in0=iota_b, in1=dcol_b, op=Alu.is_equal
                )
                return S

            def gather_group(g, src_ap, F, tag):
                """One indirect row-gather for all chunks of group g."""
                t0, tg = int(cstart[g]), int(chunks_g[g])
                gt = gp.tile([_P, TGMAX * F], f16, tag=tag)
                nc.gpsimd.indirect_dma_start(
                    out=gt[:, : tg * F],
                    out_offset=None,
                    in_=src_ap,
                    in_offset=bass.IndirectOffsetOnAxis(
                        ap=offsT[:, t0 : t0 + tg], axis=0
                    ),
                )
                return gt

            # ================= Layer 1 =================
            for g in range(_G):
                tg = int(chunks_g[g])
                gt = gather_group(g, x_d.ap(), _FIN, "g1")
                S = make_S(g)
                pa = psA.tile([_P, _P], f32, tag="agg")
                for k in range(tg):
                    nc.tensor.matmul(
                        out=pa[:],
                        lhsT=gt[:, k * _FIN : (k + 1) * _FIN],
                        rhs=S[:, k * _P : (k + 1) * _P],
                        start=(k == 0), stop=(k == tg - 1),
                    )
                ns = slice(g * _P, (g + 1) * _P)
                mean = wp.tile([_P, _P], f16, tag="mean")
                nc.vector.tensor_tensor(
                    out=mean[:], in0=pa[:], in1=ivdbc[:, ns], op=Alu.mult
                )
                for h in range(2):
                    hs = slice(h * _P, (h + 1) * _P)
                    ph = psB.tile([_P, _P], f32, tag="dense")
                    nc.tensor.matmul(out=ph[:], lhsT=w1l[:, hs], rhs=mean[:],
                                     start=True, stop=False)
                    nc.tensor.matmul(out=ph[:], lhsT=w1r[:, hs], rhs=xT[:, ns],
                                     start=False, stop=True)
                    nc.scalar.activation(out=h1T[h][:, ns], in_=ph[:],
                                         func=Act.Relu, bias=b1[:, h : h + 1])
                row = wp.tile([_P, _HID], f16, tag="row")
                for h in range(2):
                    pt = psT.tile([_P, _P], f16, tag="tr")
                    nc.tensor.transpose(out=pt[:], in_=h1T[h][:, ns],
                                        identity=ident[:])
                    nc.scalar.copy(out=row[:, h * _P : (h + 1) * _P], in_=pt[:])
                nc.sync.dma_start(out=h1_shard[ns, :], in_=row[:])

            nc.gpsimd.collective_compute(
                "AllGather", Alu.bypass, replica_groups=rg,
                ins=[h1_shard.opt()], outs=[h1_full.opt()],
            )

            # ================= Layer 2 =================
            for g in range(_G):
                tg = int(chunks_g[g])
                gt = gather_group(g, h1_full[:], _HID, "g2")
                S = make_S(g)
                pa = [psA.tile([_P, _P], f32, tag="agg", name="pa0"),
                      psT.tile([_P, _P], f32, tag="tr", name="pa1")]
                for k in range(tg):
                    Sk = S[:, k * _P : (k + 1) * _P]
                    for f in range(2):
                        nc.tensor.matmul(
                            out=pa[f][:],
                            lhsT=gt[:, k * _HID + f * _P : k * _HID + (f + 1) * _P],
                            rhs=Sk, start=(k == 0), stop=(k == tg - 1),
                        )
                ns = slice(g * _P, (g + 1) * _P)
                mean = wp.tile([_P, _HID], f16, tag="mean2")
                for f in range(2):
                    nc.vector.tensor_tensor(
                        out=mean[:, f * _P : (f + 1) * _P], in0=pa[f][:],
                        in1=ivdbc[:, ns], op=Alu.mult,
                    )
                for h in range(2):
                    hs = slice(h * _P, (h + 1) * _P)
                    ph = psB.tile([_P, _P], f32, tag="dense")
                    for f in range(2):
                        nc.tensor.matmul(out=ph[:], lhsT=w2lt[f][:, hs],
                                         rhs=mean[:, f * _P : (f + 1) * _P],
                                         start=(f == 0), stop=False)
                    for f in range(2):
                        nc.tensor.matmul(out=ph[:], lhsT=w2rt[f][:, hs],
                                         rhs=h1T[f][:, ns], start=False,
                                         stop=(f == 1))
                    nc.scalar.activation(out=h2T[h][:, ns], in_=ph[:],
                                         func=Act.Relu, bias=b2[:, h : h + 1])
                # p3 = h2 @ W3l.T  (row-major directly)
                pp = psS.tile([_P, _OUTP], f32, tag="p3")
                for f in range(2):
                    nc.tensor.matmul(out=pp[:], lhsT=h2T[f][:, ns],
                                     rhs=w3lt[f][:], start=(f == 0),
                                     stop=(f == 1))
                p3row = wp.tile([_P, _OUTP], f16, tag="p3row")
                nc.scalar.copy(out=p3row[:], in_=pp[:])
                nc.sync.dma_start(out=p3_shard[ns, :], in_=p3row[:])

            nc.gpsimd.collective_compute(
                "AllGather", Alu.bypass, replica_groups=rg,
                ins=[p3_shard.opt()], outs=[p3_full.opt()],
            )

            # ================= Layer 3 =================
            for g in range(_G):
                tg = int(chunks_g[g])
                gt = gather_group(g, p3_full[:], _OUTP, "g3")
                S = make_S(g)
                ns = slice(g * _P, (g + 1) * _P)
                poA = psS.tile([_P, _OUTP], f32, tag="p3", name="poA")
                for k in range(tg):
                    nc.tensor.matmul(
                        out=poA[:], lhsT=S[:, k * _P : (k + 1) * _P],
                        rhs=gt[:, k * _OUTP : (k + 1) * _OUTP],
                        start=(k == 0), stop=(k == tg - 1),
                    )
                poR = psB.tile([_P, _OUTP], f32, tag="dense", name="poR")
                for f in range(2):
                    nc.tensor.matmul(out=poR[:], lhsT=h2T[f][:, ns],
                                     rhs=w3rt[f][:], start=(f == 0), stop=False)
                nc.tensor.matmul(out=poR[:], lhsT=ones1[:], rhs=b3row[:],
                                 start=False, stop=True)
                # z = poA * invdeg + poR  (invdeg per dst node = per partition)
                zA = wp.tile([_P, _OUTP], f32, tag="zA")
                nc.scalar.activation(out=zA[:], in_=poA[:], func=Act.Copy,
                                     scale=ivdg[:, g : g + 1])
                nc.vector.tensor_tensor(
                    out=zbuf[:, g * _OUTP : (g + 1) * _OUTP],
                    in0=zA[:], in1=poR[:], op=Alu.add,
                )

            # ============ batched log_softmax over all groups ============
            zv = zbuf[:].rearrange("p (g f) -> p g f", g=_G)
            mx = cp.tile([_P, _G], f32, name="mx")
            nc.vector.reduce_max(mx[:], zv, axis=AxX)
            zc = cp.tile([_P, _G * _OUTP], f32, name="zc")
            nc.vector.tensor_tensor(
                out=zc[:].rearrange("p (g f) -> p g f", g=_G),
                in0=zv,
                in1=mx[:].unsqueeze(2).broadcast_to([_P, _G, _OUTP]),
                op=Alu.subtract,
            )
            ez = cp.tile([_P, _G * _OUTP], f32, name="ez")
            nc.scalar.activation(out=ez[:], in_=zc[:], func=Act.Exp)
            sm = cp.tile([_P, _G], f32, name="sm")
            nc.vector.reduce_sum(
                sm[:], ez[:].rearrange("p (g f) -> p g f", g=_G), axis=AxX
            )
            lg = cp.tile([_P, _G], f32, name="lg")
            nc.scalar.activation(out=lg[:], in_=sm[:], func=Act.Ln)
            res = cp.tile([_P, _G * _OUTP], f32, name="res")
            nc.vector.tensor_tensor(
                out=res[:].rearrange("p (g f) -> p g f", g=_G),
                in0=zc[:].rearrange("p (g f) -> p g f", g=_G),
                in1=lg[:].unsqueeze(2).broadcast_to([_P, _G, _OUTP]),
                op=Alu.subtract,
            )
            nc.sync.dma_start(
                out=out_d.ap().rearrange("(g p) f -> p g f", g=_G),
                in_=res[:].rearrange("p (g f) -> p g f", g=_G),
            )

    nc.compile()
    return nc, input_names


def _run(inputs, trace=False, tmpdir=None):
    import sys

    if "/opt/trn_rl_repo" not in sys.path:
        sys.path.insert(0, "/opt/trn_rl_repo")
    from concourse import bass_utils

    x = np.asarray(inputs["x"], np.float32)
    maps, T, chunks_g, cstart = _prep(x, inputs["edge_index"])

    x_pad = np.zeros((_NGP, _FIN), np.float16)
    for c in range(_C):
        x_pad[c * _NLP : c * _NLP + _NL] = x[c * _NL : (c + 1) * _NL]

    iota = np.tile(np.arange(_P, dtype=np.float16), (_P, 1))
    ident = np.eye(_P, dtype=np.float16)
    ones1 = np.ones((1, _P), np.float16)
    w1lT = np.ascontiguousarray(np.asarray(inputs["W1l"], np.float32).T).astype(np.float16)
    w1rT = np.ascontiguousarray(np.asarray(inputs["W1r"], np.float32).T).astype(np.float16)
    w2lT = np.ascontiguousarray(np.asarray(inputs["W2l"], np.float32).T).astype(np.float16)
    w2rT = np.ascontiguousarray(np.asarray(inputs["W2r"], np.float32).T).astype(np.float16)
    w3lT = np.zeros((_HID, _OUTP), np.float16)
    w3lT[:, :_OUT] = np.asarray(inputs["W3l"], np.float32).T
    w3rT = np.zeros((_HID, _OUTP), np.float16)
    w3rT[:, :_OUT] = np.asarray(inputs["W3r"], np.float32).T
    b1 = np.ascontiguousarray(
        np.asarray(inputs["b1l"], np.float32).reshape(2, _P).T)
    b2 = np.ascontiguousarray(
        np.asarray(inputs["b2l"], np.float32).reshape(2, _P).T)
    b3row = np.full((1, _OUTP), -30000.0, np.float16)
    b3row[0, :_OUT] = np.asarray(inputs["b3l"], np.float32)

    shared = dict(
        xpad=x_pad, iota=iota, ident=ident, ones1=ones1,
        w1lT=w1lT, w1rT=w1rT, w2lT=w2lT, w2rT=w2rT, w3lT=w3lT, w3rT=w3rT,
        b1=b1, b2=b2, b3row=b3row,
    )
    in_maps = []
    for c in range(_C):
        m = dict(shared)
        m["xT"] = maps[c]["xT"]
        m["offs"] = maps[c]["offs"]
        m["dcol"] = maps[c]["dcol"]
        m["ivdbc"] = maps[c]["ivdbc"]
        m["ivdg"] = maps[c]["ivdg"]
        in_maps.append(m)

    nc, input_names = _build(T, chunks_g, cstart)

    res = bass_utils.run_bass_kernel_spmd(
        nc, in_maps, core_ids=list(range(_C)), trace=trace, tmpdir=tmpdir,
    )
    outs = res.results
    y = np.concatenate(
        [np.asarray(outs[c]["out"])[:_NL, :_OUT] for c in range(_C)], axis=0
    ).astype(np.float32)
    return y, res


def kernel(**inputs):
    y, _ = _run(inputs, trace=False)
    return y
